# revision 5
# baseline (speedup 1.0000x reference)
"""Distributed Trainium2 kernel for DeepseekV3-style GQA attention.

Problem (hardcoded): B=1, S=4096, H=2048, NQ=16 q heads, NKV=4 kv heads,
D=128, rotate-half RoPE (theta=10000) over full head dim, causal softmax,
o_proj. 8 NeuronCores, tensor-parallel over heads:

  core c: q heads {2c, 2c+1}, kv head c//2 (replicated across the pair),
  Wq/Wk/Wv column-sharded, attention computed flash-style in bf16 with
  f32 PSUM accumulation, attention output produced transposed [j, s],
  AllGathered in 4 sequence chunks (overlapped with attention compute),
  o_proj column-sharded (Wo[:, c*256:(c+1)*256]) -> out shard [S, 256].

Host side: casts inputs to bf16, pre-transposes x, builds cos/sin tables
from position_ids, shards weights, concatenates output shards.
"""
import os
import sys

sys.path.insert(0, "/opt/trn_rl_repo")

import numpy as np
import ml_dtypes

import concourse.bass as bass
import concourse.bacc as bacc
import concourse.mybir as mybir
import concourse.tile as tile
from concourse.bass_utils import run_bass_kernel_spmd

BF16 = mybir.dt.bfloat16
F32 = mybir.dt.float32
NPBF16 = ml_dtypes.bfloat16

B, S, H = 1, 4096, 2048
NQ, NKV, D = 16, 4, 128
THETA = 10000.0
NCORES = 8
HPC = NQ // NCORES          # q heads per core = 2
OC = H // NCORES            # o_proj out cols per core = 256
SC = 512                    # projection s-chunk
NSC = S // SC               # 8
NKT = S // 128              # 32 k tiles of 128
QS = 512                    # attention q supertile
NQS = S // QS               # 8
CHUNK = 1024                # allgather s-chunk
NCH = S // CHUNK            # 4
SCALE = 1.0 / float(np.sqrt(D))

_cached = {}


def _build():
    nc = bacc.Bacc("TRN2", target_bir_lowering=False, debug=False,
                   num_devices=NCORES)

    xT = nc.declare_dram_parameter("xT", [H, S], BF16, isOutput=False)
    wq = nc.declare_dram_parameter("wq", [H, HPC * D], BF16, isOutput=False)
    wk = nc.declare_dram_parameter("wk", [H, D], BF16, isOutput=False)
    wv = nc.declare_dram_parameter("wv", [H, D], BF16, isOutput=False)
    wo = nc.declare_dram_parameter("wo", [H, OC], BF16, isOutput=False)
    cosT = nc.declare_dram_parameter("cosT", [D, S], BF16, isOutput=False)
    sinT = nc.declare_dram_parameter("sinT", [D, S], BF16, isOutput=False)
    trimask = nc.declare_dram_parameter("trimask", [128, 128], BF16, isOutput=False)
    identity = nc.declare_dram_parameter("identity", [128, 128], BF16, isOutput=False)
    out = nc.declare_dram_parameter("out", [S, OC], F32, isOutput=True)

    with tile.TileContext(nc) as tc:
        with (
            tc.tile_pool(name="const", bufs=1) as constp,
            tc.tile_pool(name="persist", bufs=1) as persist,
            tc.tile_pool(name="xtp", bufs=24) as xtp,
            tc.tile_pool(name="ropep", bufs=2) as ropep,
            tc.tile_pool(name="ptp", bufs=3) as ptp,
            tc.tile_pool(name="attnp", bufs=2) as attnp,
            tc.tile_pool(name="smallp", bufs=3) as smallp,
            tc.tile_pool(name="agp", bufs=16) as agp,
            tc.tile_pool(name="outp", bufs=3) as outp,
            tc.tile_pool(name="dram", bufs=1, space="DRAM") as dramp,
        ):
            # ---- constants / persistent tensors ----
            tri = constp.tile([128, 128], BF16, tag="tri")
            nc.sync.dma_start(tri[:], trimask[:])
            ident = constp.tile([128, 128], BF16, tag="ident")
            nc.sync.dma_start(ident[:], identity[:])
            cos_sb = persist.tile([128, S], BF16, tag="cos")
            nc.sync.dma_start(cos_sb[:], cosT[:])
            sin_sb = persist.tile([128, S], BF16, tag="sin")
            nc.sync.dma_start(sin_sb[:], sinT[:])

            wq_sb, wk_sb, wv_sb, wo_sb = [], [], [], []
            for t in range(16):
                wqt = persist.tile([128, HPC * D], BF16, tag=f"wq{t}")
                nc.sync.dma_start(wqt[:], wq[t * 128:(t + 1) * 128, :])
                wq_sb.append(wqt)
                wkt = persist.tile([128, D], BF16, tag=f"wk{t}")
                nc.sync.dma_start(wkt[:], wk[t * 128:(t + 1) * 128, :])
                wk_sb.append(wkt)
                wvt = persist.tile([128, D], BF16, tag=f"wv{t}")
                nc.sync.dma_start(wvt[:], wv[t * 128:(t + 1) * 128, :])
                wv_sb.append(wvt)
                wot = persist.tile([128, OC], BF16, tag=f"wo{t}")
                nc.sync.dma_start(wot[:], wo[t * 128:(t + 1) * 128, :])
                wo_sb.append(wot)

            QT = [persist.tile([128, S], BF16, tag=f"qt{h}", name=f"qt{h}")
                  for h in range(HPC)]
            KT = persist.tile([128, S], BF16, tag="kt")
            V = []
            for kt in range(NKT):
                vt = persist.tile([128, D + 1], BF16, tag=f"v{kt}")
                nc.gpsimd.memset(vt[:, D:D + 1], 1.0)
                V.append(vt)

            # ---- phase 1: projections (chunked over s) + RoPE ----
            with (
                tc.tile_pool(name="projps", bufs=4, space="PSUM") as projps,
                tc.tile_pool(name="vps", bufs=2, space="PSUM") as vps,
            ):
                for sc in range(NSC):
                    off = sc * SC
                    xt = []
                    for t in range(16):
                        x_t = xtp.tile([128, SC], BF16, tag="xt")
                        nc.sync.dma_start(x_t[:], xT[t * 128:(t + 1) * 128,
                                                     off:off + SC])
                        xt.append(x_t)

                    # q/k projections -> transposed layout [d, s]
                    def project_rope(lhs_cols, dst):
                        ps = projps.tile([128, SC], F32, tag="proj")
                        for t in range(16):
                            nc.tensor.matmul(ps[:], lhs_cols[t], xt[t][:],
                                             start=(t == 0), stop=(t == 15))
                        raw = ropep.tile([128, SC], BF16, tag="raw")
                        nc.vector.tensor_copy(raw[:], ps[:])
                        sw = ropep.tile([128, SC], BF16, tag="sw")
                        nc.sync.dma_start(sw[0:64, :], raw[64:128, :])
                        nc.sync.dma_start(sw[64:128, :], raw[0:64, :])
                        t1 = ropep.tile([128, SC], BF16, tag="t1")
                        nc.vector.tensor_tensor(t1[:], raw[:],
                                                cos_sb[:, off:off + SC],
                                                mybir.AluOpType.mult)
                        t2 = ropep.tile([128, SC], BF16, tag="t2")
                        nc.vector.tensor_tensor(t2[:], sw[:],
                                                sin_sb[:, off:off + SC],
                                                mybir.AluOpType.mult)
                        nc.vector.tensor_tensor(dst[:, off:off + SC], t1[:],
                                                t2[:], mybir.AluOpType.add)

                    for h in range(HPC):
                        project_rope([w[:, h * D:(h + 1) * D] for w in wq_sb],
                                     QT[h])
                    project_rope([w[:] for w in wk_sb], KT)

                    # v projection (natural [s, d] layout)
                    for st in range(SC // 128):
                        v_ps = vps.tile([128, 128], F32, tag="vps")
                        for t in range(16):
                            nc.tensor.matmul(
                                v_ps[:], xt[t][:, st * 128:(st + 1) * 128],
                                wv_sb[t][:], start=(t == 0), stop=(t == 15))
                        kti = sc * (SC // 128) + st
                        nc.vector.tensor_copy(V[kti][:, 0:D], v_ps[:])

            # ---- phases 2+3 interleaved: attention, AG, o_proj ----
            with (
                tc.tile_pool(name="stps", bufs=2, space="PSUM") as stps,
                tc.tile_pool(name="attps", bufs=4, space="PSUM") as attps,
                tc.tile_pool(name="smps", bufs=2, space="PSUM") as smps,
            ):
                attnT_cur = [None, None]
                bounces = []

                def attention(head, qs):
                    q_off = qs * QS
                    att = [attps.tile([128, D + 1], F32, tag="att", name=f"att{i}")
                           for i in range(4)]
                    nkt = 4 * qs + 4
                    pend = None  # (kt, j, q_lo, pt)
                    for kt in range(nkt):
                        j = kt - 4 * qs
                        q_lo = 128 * j if j > 0 else 0
                        N = QS - q_lo
                        st_ps = stps.tile([128, QS], F32, tag="st")
                        nc.tensor.matmul(
                            st_ps[:, 0:N],
                            KT[:, kt * 128:(kt + 1) * 128],
                            QT[head][:, q_off + q_lo:q_off + QS],
                            start=True, stop=True)
                        pt = ptp.tile([128, QS], BF16, tag="pt")
                        nc.scalar.activation(pt[:, 0:N], st_ps[:, 0:N],
                                             mybir.ActivationFunctionType.Exp,
                                             scale=SCALE)
                        if j >= 0:
                            nc.vector.tensor_tensor(pt[:, 0:128], pt[:, 0:128],
                                                    tri[:],
                                                    mybir.AluOpType.mult)
                        if pend is not None:
                            _attv(*pend, att, qs)
                        pend = (kt, j, q_lo, pt)
                    _attv(*pend, att, qs)

                    # normalize + transpose into attnT
                    for qsub in range(4):
                        recip = smallp.tile([128, 1], F32, tag="recip")
                        nc.vector.reciprocal(recip[:], att[qsub][:, D:D + 1])
                        attn_n = smallp.tile([128, 128], BF16, tag="attn_n")
                        nc.vector.tensor_scalar(attn_n[:], att[qsub][:, 0:D],
                                                recip[:], None,
                                                mybir.AluOpType.mult)
                        tr = smps.tile([128, 128], BF16, tag="tr")
                        nc.tensor.transpose(tr[:], attn_n[:], ident[:])
                        col = (qs % 2) * QS + qsub * 128
                        nc.vector.tensor_copy(
                            attnT_cur[head][:, col:col + 128], tr[:])

                def _attv(kt, j, q_lo, pt, att, qs):
                    for qsub in range(max(j, 0), 4):
                        nc.tensor.matmul(
                            att[qsub][:],
                            pt[:, qsub * 128 - q_lo:qsub * 128 - q_lo + 128],
                            V[kt][:],
                            start=(kt == 0), stop=(kt == 4 * qs + qsub))

                def emit_ag(ci):
                    bounce = dramp.tile([2 * 128, CHUNK], BF16, tag=f"bn{ci}")
                    for head in range(HPC):
                        nc.sync.dma_start(
                            bounce[head * 128:(head + 1) * 128, :],
                            attnT_cur[head][:])
                    gathered = dramp.tile([NCORES * 2 * 128, CHUNK], BF16,
                                          tag=f"ag{ci}", addr_space="Shared")
                    nc.gpsimd.collective_compute(
                        "AllGather", mybir.AluOpType.bypass,
                        replica_groups=[list(range(NCORES))],
                        ins=[bounce.opt()], outs=[gathered.opt()])
                    bounces.append(gathered)

                def emit_oproj(ci):
                    gathered = bounces[ci]
                    ag_sb = []
                    for jt in range(16):
                        a = agp.tile([128, CHUNK], BF16, tag="ag")
                        nc.sync.dma_start(a[:], gathered[jt * 128:(jt + 1) * 128, :])
                        ag_sb.append(a)
                    for sb_i in range(CHUNK // 128):
                        o_ps = smps.tile([128, OC], F32, tag="tr", name="o_ps")
                        for jt in range(16):
                            nc.tensor.matmul(
                                o_ps[:],
                                ag_sb[jt][:, sb_i * 128:(sb_i + 1) * 128],
                                wo_sb[jt][:], start=(jt == 0), stop=(jt == 15))
                        o_sb = outp.tile([128, OC], F32, tag="osb")
                        nc.vector.tensor_copy(o_sb[:], o_ps[:])
                        row = ci * CHUNK + sb_i * 128
                        nc.sync.dma_start(out[row:row + 128, :], o_sb[:])

                for qs in range(NQS):
                    if qs % 2 == 0:
                        for head in range(HPC):
                            attnT_cur[head] = attnp.tile(
                                [128, CHUNK], BF16, tag=f"attnT{head}",
                                name=f"attnT{head}_{qs}")
                    for head in range(HPC):
                        attention(head, qs)
                    if qs % 2 == 1:
                        ci = qs // 2
                        emit_ag(ci)
                        if ci >= 1:
                            emit_oproj(ci - 1)
                emit_oproj(NCH - 1)

    nc.compile()
    return nc


def _get_nc():
    if "nc" not in _cached:
        _cached["nc"] = _build()
    return _cached["nc"]


def _prep_inputs(hidden_states, Wq, Wk, Wv, Wo, position_ids):
    x = np.asarray(hidden_states, dtype=np.float32).reshape(S, H)
    xT = np.ascontiguousarray(x.T).astype(NPBF16)
    Wq = np.asarray(Wq, dtype=np.float32)
    Wk = np.asarray(Wk, dtype=np.float32)
    Wv = np.asarray(Wv, dtype=np.float32)
    Wo = np.asarray(Wo, dtype=np.float32)
    pos = np.asarray(position_ids).reshape(S).astype(np.float32)

    half = D // 2
    inv_freq = 1.0 / (THETA ** (np.arange(half, dtype=np.float32) * 2.0 / D))
    freqs = inv_freq[:, None] * pos[None, :]          # [64, S]
    c64 = np.cos(freqs, dtype=np.float32)
    s64 = np.sin(freqs, dtype=np.float32)
    cosT = np.vstack([c64, c64]).astype(NPBF16)       # [128, S]
    sinT = np.vstack([-s64, s64]).astype(NPBF16)      # signed for rotate-half
    tri = np.triu(np.ones((128, 128), dtype=np.float32)).astype(NPBF16)
    ident = np.eye(128, dtype=np.float32).astype(NPBF16)

    in_maps = []
    for c in range(NCORES):
        kvh = c // 2
        in_maps.append({
            "xT": xT,
            "wq": np.ascontiguousarray(Wq[:, c * HPC * D:(c + 1) * HPC * D]).astype(NPBF16),
            "wk": np.ascontiguousarray(Wk[:, kvh * D:(kvh + 1) * D]).astype(NPBF16),
            "wv": np.ascontiguousarray(Wv[:, kvh * D:(kvh + 1) * D]).astype(NPBF16),
            "wo": np.ascontiguousarray(Wo[:, c * OC:(c + 1) * OC]).astype(NPBF16),
            "cosT": cosT,
            "sinT": sinT,
            "trimask": tri,
            "identity": ident,
        })
    return in_maps


def _run(inputs, trace=False):
    nc = _get_nc()
    in_maps = _prep_inputs(**inputs)
    res = run_bass_kernel_spmd(nc, in_maps, list(range(NCORES)), trace=trace)
    shards = [res.results[c]["out"] for c in range(NCORES)]
    full = np.concatenate(shards, axis=1).reshape(B, S, H).astype(np.float32)
    return full, res


def kernel(**inputs):
    full, _ = _run(inputs, trace=False)
    return full


# revision 6
# speedup vs baseline: 1.0076x; 1.0076x over previous
"""Distributed Trainium2 kernel for DeepseekV3-style GQA attention.

Problem (hardcoded): B=1, S=4096, H=2048, NQ=16 q heads, NKV=4 kv heads,
D=128, rotate-half RoPE (theta=10000) over full head dim, causal softmax,
o_proj. 8 NeuronCores, tensor-parallel over heads:

  core c: q heads {2c, 2c+1}, kv head c//2 (replicated across the pair),
  Wq/Wk/Wv column-sharded, attention computed flash-style in bf16 with
  f32 PSUM accumulation, attention output produced transposed [j, s],
  AllGathered in 4 sequence chunks (overlapped with attention compute),
  o_proj column-sharded (Wo[:, c*256:(c+1)*256]) -> outT shard [256, S].

Host side: casts inputs to bf16, pre-transposes x, builds cos/sin tables
from position_ids, shards weights, transposes + concatenates out shards.
"""
import os
import sys

sys.path.insert(0, "/opt/trn_rl_repo")

import numpy as np
import ml_dtypes

import concourse.bass as bass
import concourse.bacc as bacc
import concourse.mybir as mybir
import concourse.tile as tile
from concourse.bass_utils import run_bass_kernel_spmd

BF16 = mybir.dt.bfloat16
F32 = mybir.dt.float32
NPBF16 = ml_dtypes.bfloat16

B, S, H = 1, 4096, 2048
NQ, NKV, D = 16, 4, 128
THETA = 10000.0
NCORES = 8
HPC = NQ // NCORES          # q heads per core = 2
OC = H // NCORES            # o_proj out cols per core = 256
SC = 512                    # projection s-chunk
NSC = S // SC               # 8
NKT = S // 128              # 32 k tiles of 128
QS = 512                    # attention q supertile
NQS = S // QS               # 8
CHUNK = 1024                # allgather s-chunk
NCH = S // CHUNK            # 4
SCALE = 1.0 / float(np.sqrt(D))

_cached = {}


def _build():
    nc = bacc.Bacc("TRN2", target_bir_lowering=False, debug=False,
                   num_devices=NCORES)

    xT = nc.declare_dram_parameter("xT", [H, S], BF16, isOutput=False)
    wq = nc.declare_dram_parameter("wq", [H, HPC * D], BF16, isOutput=False)
    wk = nc.declare_dram_parameter("wk", [H, D], BF16, isOutput=False)
    wv = nc.declare_dram_parameter("wv", [H, D], BF16, isOutput=False)
    wo = nc.declare_dram_parameter("wo", [H, OC], BF16, isOutput=False)
    cosT = nc.declare_dram_parameter("cosT", [D, S], BF16, isOutput=False)
    sinT = nc.declare_dram_parameter("sinT", [D, S], BF16, isOutput=False)
    trimask = nc.declare_dram_parameter("trimask", [128, 128], BF16, isOutput=False)
    identity = nc.declare_dram_parameter("identity", [128, 128], BF16, isOutput=False)
    out = nc.declare_dram_parameter("out", [OC, S], F32, isOutput=True)

    with tile.TileContext(nc) as tc:
        with (
            tc.tile_pool(name="const", bufs=1) as constp,
            tc.tile_pool(name="persist", bufs=1) as persist,
            tc.tile_pool(name="xtp", bufs=2) as xtp,
            tc.tile_pool(name="ropep", bufs=2) as ropep,
            tc.tile_pool(name="ptp", bufs=3) as ptp,
            tc.tile_pool(name="attnp", bufs=3) as attnp,
            tc.tile_pool(name="smallp", bufs=3) as smallp,
            tc.tile_pool(name="agp", bufs=1) as agp,
            tc.tile_pool(name="outp", bufs=2) as outp,
            tc.tile_pool(name="dram", bufs=1, space="DRAM") as dramp,
        ):
            # ---- weights first (first matmuls need wq + xt chunk 0) ----
            wq_all = persist.tile([128, 16 * HPC * D], BF16, tag="wq")
            nc.sync.dma_start(wq_all[:].rearrange("p (t d) -> p t d", t=16),
                              wq[:].rearrange("(t p) d -> p t d", p=128))
            wk_all = persist.tile([128, 16 * D], BF16, tag="wk")
            nc.sync.dma_start(wk_all[:].rearrange("p (t d) -> p t d", t=16),
                              wk[:].rearrange("(t p) d -> p t d", p=128))
            wv_all = persist.tile([128, 16 * D], BF16, tag="wv")
            nc.sync.dma_start(wv_all[:].rearrange("p (t d) -> p t d", t=16),
                              wv[:].rearrange("(t p) d -> p t d", p=128))

            cos_sb = persist.tile([128, S], BF16, tag="cos")
            nc.sync.dma_start(cos_sb[:], cosT[:])
            sin_sb = persist.tile([128, S], BF16, tag="sin")
            nc.sync.dma_start(sin_sb[:], sinT[:])
            tri = constp.tile([128, 128], BF16, tag="tri")
            nc.sync.dma_start(tri[:], trimask[:])
            ident = constp.tile([128, 128], BF16, tag="ident")
            nc.sync.dma_start(ident[:], identity[:])
            wo_all = persist.tile([128, 16 * OC], BF16, tag="wo")
            nc.scalar.dma_start(wo_all[:].rearrange("p (t d) -> p t d", t=16),
                                wo[:].rearrange("(t p) d -> p t d", p=128))

            def wqt(t, h):
                return wq_all[:, t * HPC * D + h * D: t * HPC * D + (h + 1) * D]

            QT = [persist.tile([128, S], BF16, tag=f"qt{h}", name=f"qt{h}")
                  for h in range(HPC)]
            KT = persist.tile([128, S], BF16, tag="kt")
            V = []
            for kt in range(NKT):
                vt = persist.tile([128, D + 1], BF16, tag=f"v{kt}")
                nc.gpsimd.memset(vt[:, D:D + 1], 1.0)
                V.append(vt)

            # ---- phase 1: projections (chunked over s) + RoPE ----
            with (
                tc.tile_pool(name="projps", bufs=4, space="PSUM") as projps,
                tc.tile_pool(name="vps", bufs=2, space="PSUM") as vps,
            ):
                for sc in range(NSC):
                    off = sc * SC
                    xt_all = xtp.tile([128, 16 * SC], BF16, tag="xt",
                                      name="xt_all")
                    nc.sync.dma_start(
                        xt_all[:].rearrange("p (t s) -> p t s", t=16),
                        xT[:, off:off + SC].rearrange("(t p) s -> p t s", p=128))

                    def xts(t):
                        return xt_all[:, t * SC:(t + 1) * SC]

                    # q/k projections -> transposed layout [d, s]
                    def project_rope(lhs_of_t, dst):
                        ps = projps.tile([128, SC], F32, tag="proj", name="ps")
                        for t in range(16):
                            nc.tensor.matmul(ps[:], lhs_of_t(t), xts(t),
                                             start=(t == 0), stop=(t == 15))
                        raw = ropep.tile([128, SC], BF16, tag="raw", name="raw")
                        nc.vector.tensor_copy(raw[:], ps[:])
                        sw = ropep.tile([128, SC], BF16, tag="sw", name="sw")
                        nc.sync.dma_start(sw[0:64, :], raw[64:128, :])
                        nc.sync.dma_start(sw[64:128, :], raw[0:64, :])
                        t1 = ropep.tile([128, SC], BF16, tag="t1", name="t1")
                        nc.vector.tensor_tensor(t1[:], raw[:],
                                                cos_sb[:, off:off + SC],
                                                mybir.AluOpType.mult)
                        t2 = ropep.tile([128, SC], BF16, tag="t2", name="t2")
                        nc.vector.tensor_tensor(t2[:], sw[:],
                                                sin_sb[:, off:off + SC],
                                                mybir.AluOpType.mult)
                        nc.vector.tensor_tensor(dst[:, off:off + SC], t1[:],
                                                t2[:], mybir.AluOpType.add)

                    for h in range(HPC):
                        project_rope(lambda t, h=h: wqt(t, h), QT[h])
                    project_rope(lambda t: wk_all[:, t * D:(t + 1) * D], KT)

                    # v projection (natural [s, d] layout)
                    for st in range(SC // 128):
                        v_ps = vps.tile([128, 128], F32, tag="vps", name="v_ps")
                        for t in range(16):
                            nc.tensor.matmul(
                                v_ps[:], xt_all[:, t * SC + st * 128:
                                                t * SC + (st + 1) * 128],
                                wv_all[:, t * D:(t + 1) * D],
                                start=(t == 0), stop=(t == 15))
                        kti = sc * (SC // 128) + st
                        nc.vector.tensor_copy(V[kti][:, 0:D], v_ps[:])

            # ---- phases 2+3 interleaved: attention, AG, o_proj ----
            with (
                tc.tile_pool(name="stps", bufs=2, space="PSUM") as stps,
                tc.tile_pool(name="attps", bufs=4, space="PSUM") as attps,
                tc.tile_pool(name="smps", bufs=2, space="PSUM") as smps,
            ):
                attnT_cur = [None, None]
                bounces = []

                def _attv(kt, j, q_lo, pt, att, qs):
                    for qsub in range(max(j, 0), 4):
                        nc.tensor.matmul(
                            att[qsub][:],
                            pt[:, qsub * 128 - q_lo:qsub * 128 - q_lo + 128],
                            V[kt][:],
                            start=(kt == 0), stop=(kt == 4 * qs + qsub))

                def attention(head, qs):
                    q_off = qs * QS
                    att = [attps.tile([128, D + 1], F32, tag="att", name=f"att{i}")
                           for i in range(4)]
                    nkt = 4 * qs + 4
                    pend = None  # (kt, j, q_lo, pt)
                    for kt in range(nkt):
                        j = kt - 4 * qs
                        q_lo = 128 * j if j > 0 else 0
                        N = QS - q_lo
                        st_ps = stps.tile([128, QS], F32, tag="st", name="st_ps")
                        nc.tensor.matmul(
                            st_ps[:, 0:N],
                            KT[:, kt * 128:(kt + 1) * 128],
                            QT[head][:, q_off + q_lo:q_off + QS],
                            start=True, stop=True)
                        pt = ptp.tile([128, QS], BF16, tag="pt", name="pt")
                        nc.scalar.activation(pt[:, 0:N], st_ps[:, 0:N],
                                             mybir.ActivationFunctionType.Exp,
                                             scale=SCALE)
                        if j >= 0:
                            nc.vector.tensor_tensor(pt[:, 0:128], pt[:, 0:128],
                                                    tri[:],
                                                    mybir.AluOpType.mult)
                        if pend is not None:
                            _attv(*pend, att, qs)
                        pend = (kt, j, q_lo, pt)
                    _attv(*pend, att, qs)

                    # normalize + transpose into attnT
                    for qsub in range(4):
                        recip = smallp.tile([128, 1], F32, tag="recip",
                                            name="recip")
                        nc.vector.reciprocal(recip[:], att[qsub][:, D:D + 1])
                        attn_n = smallp.tile([128, 128], BF16, tag="attn_n",
                                             name="attn_n")
                        nc.vector.tensor_scalar(attn_n[:], att[qsub][:, 0:D],
                                                recip[:], None,
                                                mybir.AluOpType.mult)
                        tr = smps.tile([128, 128], BF16, tag="tr", name="tr")
                        nc.tensor.transpose(tr[:], attn_n[:], ident[:])
                        col = (qs % 2) * QS + qsub * 128
                        nc.vector.tensor_copy(
                            attnT_cur[head][:, col:col + 128], tr[:])

                def emit_ag(ci):
                    bounce = dramp.tile([2 * 128, CHUNK], BF16, tag=f"bn{ci}",
                                        name=f"bn{ci}")
                    for head in range(HPC):
                        nc.scalar.dma_start(
                            bounce[head * 128:(head + 1) * 128, :],
                            attnT_cur[head][:])
                    gathered = dramp.tile([NCORES * 2 * 128, CHUNK], BF16,
                                          tag=f"ag{ci}", name=f"ag{ci}",
                                          addr_space="Shared")
                    nc.gpsimd.collective_compute(
                        "AllGather", mybir.AluOpType.bypass,
                        replica_groups=[list(range(NCORES))],
                        ins=[bounce.opt()], outs=[gathered.opt()])
                    bounces.append(gathered)

                def emit_oproj(ci):
                    gathered = bounces[ci]
                    ag_all = agp.tile([128, 16 * CHUNK], BF16, tag="ag",
                                      name="ag_all")
                    nc.scalar.dma_start(
                        ag_all[:].rearrange("p (t s) -> p t s", t=16),
                        gathered[:].rearrange("(t p) s -> p t s", p=128))
                    for oct_ in range(OC // 128):
                        o_sb = outp.tile([128, CHUNK], F32, tag="osb",
                                         name="o_sb")
                        for sti in range(CHUNK // 512):
                            o_ps = smps.tile([128, 512], F32, tag="tr",
                                             name="o_ps")
                            for jt in range(16):
                                nc.tensor.matmul(
                                    o_ps[:],
                                    wo_all[:, jt * OC + oct_ * 128:
                                           jt * OC + oct_ * 128 + 128],
                                    ag_all[:, jt * CHUNK + sti * 512:
                                           jt * CHUNK + (sti + 1) * 512],
                                    start=(jt == 0), stop=(jt == 15))
                            nc.vector.tensor_copy(
                                o_sb[:, sti * 512:(sti + 1) * 512], o_ps[:])
                        nc.scalar.dma_start(
                            out[oct_ * 128:(oct_ + 1) * 128,
                                ci * CHUNK:(ci + 1) * CHUNK], o_sb[:])

                for qs in range(NQS):
                    if qs % 2 == 0:
                        for head in range(HPC):
                            attnT_cur[head] = attnp.tile(
                                [128, CHUNK], BF16, tag=f"attnT{head}",
                                name=f"attnT{head}_{qs}")
                    for head in range(HPC):
                        attention(head, qs)
                    if qs % 2 == 1:
                        ci = qs // 2
                        emit_ag(ci)
                        if ci >= 1:
                            emit_oproj(ci - 1)
                emit_oproj(NCH - 1)

    nc.compile()
    return nc


def _get_nc():
    if "nc" not in _cached:
        _cached["nc"] = _build()
    return _cached["nc"]


def _prep_inputs(hidden_states, Wq, Wk, Wv, Wo, position_ids):
    x = np.asarray(hidden_states, dtype=np.float32).reshape(S, H)
    xT = np.ascontiguousarray(x.T).astype(NPBF16)
    Wq = np.asarray(Wq, dtype=np.float32)
    Wk = np.asarray(Wk, dtype=np.float32)
    Wv = np.asarray(Wv, dtype=np.float32)
    Wo = np.asarray(Wo, dtype=np.float32)
    pos = np.asarray(position_ids).reshape(S).astype(np.float32)

    half = D // 2
    inv_freq = 1.0 / (THETA ** (np.arange(half, dtype=np.float32) * 2.0 / D))
    freqs = inv_freq[:, None] * pos[None, :]          # [64, S]
    c64 = np.cos(freqs, dtype=np.float32)
    s64 = np.sin(freqs, dtype=np.float32)
    cosT = np.vstack([c64, c64]).astype(NPBF16)       # [128, S]
    sinT = np.vstack([-s64, s64]).astype(NPBF16)      # signed for rotate-half
    tri = np.triu(np.ones((128, 128), dtype=np.float32)).astype(NPBF16)
    ident = np.eye(128, dtype=np.float32).astype(NPBF16)

    in_maps = []
    for c in range(NCORES):
        kvh = c // 2
        in_maps.append({
            "xT": xT,
            "wq": np.ascontiguousarray(Wq[:, c * HPC * D:(c + 1) * HPC * D]).astype(NPBF16),
            "wk": np.ascontiguousarray(Wk[:, kvh * D:(kvh + 1) * D]).astype(NPBF16),
            "wv": np.ascontiguousarray(Wv[:, kvh * D:(kvh + 1) * D]).astype(NPBF16),
            "wo": np.ascontiguousarray(Wo[:, c * OC:(c + 1) * OC]).astype(NPBF16),
            "cosT": cosT,
            "sinT": sinT,
            "trimask": tri,
            "identity": ident,
        })
    return in_maps


def _run(inputs, trace=False):
    nc = _get_nc()
    in_maps = _prep_inputs(**inputs)
    res = run_bass_kernel_spmd(nc, in_maps, list(range(NCORES)), trace=trace)
    shards = [np.ascontiguousarray(res.results[c]["out"].T)
              for c in range(NCORES)]
    full = np.concatenate(shards, axis=1).reshape(B, S, H).astype(np.float32)
    return full, res


def kernel(**inputs):
    full, _ = _run(inputs, trace=False)
    return full
